# revision 104
# baseline (speedup 1.0000x reference)
"""Multi-head attention (B=4, S=2048, H=8, Dh=64, Dm=512) on 8 TRN2 NeuronCores.

Sharding: batch*head parallel. Core c owns batch b = c//2 and head group
g = c%2 (4 heads each). Each core computes QKV projection for its head
group, transposed-scores flash-style attention (no max subtraction --
scores ~ N(0,1) after 1/sqrt(Dh) scaling, exp is safe in fp32/bf16), and
its partial output projection against its 256 rows of Wo. The host sums
the two partial projections per batch.

The exp work is SPLIT between ScalarE (exact table exp) and the Vector
engine (one-op Schraudolph fast-exp: int16(s*A+B) bit-pattern IS the bf16
of exp(s*SCALE), ~2% RMS rel err, final output rel err ~6e-3 vs the 2e-2
gate). With exp off the single-engine critical path the kernel is PE-bound
at the bf16 roofline (~150us of matmul busy; fp8/DoubleRow AV and fp8 QKV
were measured and rejected: fp8 QKV fails numerics outright, fp8-P AV is a
net perf loss because ACT fp8-output runs 1.3us/tile vs 1.1 and DR matmuls
at ~700ns vs 2x450 regular).

Schedule (as originally designed, ScalarE-centric; now PE-bound):
  - X^T (bf16) prepared on host; every matmul contracts over partitions.
  - Scores computed transposed (S^T = K Q^T); the two heads of a 128-row
    chunk run as two concurrent K=64 PE row-tiles (auto tile_position
    from the lhsT base partition).
  - Minimal lead (Q chunk 0 + K chunk 0 only) with DMA ordered by first
    use and warmup matmuls interleaved to fill the DMA-wait gaps (keeps
    the HAM activity window fed). All remaining Q/K chunks stream at one
    matmul per j-slot; V chunks and the pair-0 K chunks fill block 0.
  - AV matmuls lag exp by 2 (h0) / 3 (h1) iterations so the in-order PE
    queue never blocks on the exp semaphore; the next block's first
    scores+exp are emitted before the AV tail at each block boundary.
  - Row sums of exp come from a ones-column appended to V (M=65
    stationary); normalization = fp16 K=1 broadcast matmul + DVE
    fast-reciprocal/multiply, emitted lazily into the next block.
"""

import os
import sys

for _p in ("/opt/trn_rl_repo",):
    if os.path.isdir(_p) and _p not in sys.path:
        sys.path.append(_p)

import ml_dtypes
import numpy as np

import concourse.bass as bass
import concourse.tile as tile
from concourse import bacc, mybir
from concourse.bass_utils import run_bass_kernel_spmd

BF16 = mybir.dt.bfloat16
F16 = mybir.dt.float16
F32 = mybir.dt.float32
FP8 = mybir.dt.float8e4
I16 = mybir.dt.int16

B, S, DM = 4, 2048, 512
H, DH = 8, 64
HPC = 4  # heads per core
DQ = HPC * DH  # 256: per-core slice of the inner dim
N_CORES = 8
SCALE = DH**-0.5

AF = mybir.ActivationFunctionType

# Max fp8 DoubleRow AV pairs per block. Measured on HW: fp8 output slows
# ACT exp 1.1->1.3us and the DR matmuls land at ~700ns vs 2x450 regular, so
# fp8 is a net loss here -- keep 0 (pure bf16) unless that changes.
MAX_FP8_PAIRS = 0
USE_FP8 = MAX_FP8_PAIRS > 0

# Global exp shift (needed only to keep fp8 P-tiles under e4m3's 448 max).
# Numerator and denominator share the factor, so the softmax ratio is exact.
EXP_SHIFT = -4.0 if USE_FP8 else 0.0

# Schraudolph fast-exp constants (DVE path): exp(s*SCALE + SHIFT) ~=
# bf16_bits(int16(s * A_EXP + B_EXP)). A = SCALE * 128/ln2; B = 127*128 - C
# + SHIFT*128/ln2, C ~ 5.5 tuned for min RMS rel err (~2%).
A_EXP = SCALE * 128.0 / float(np.log(2.0))
B_EXP = 16256.0 - 5.5 + EXP_SHIFT * 128.0 / float(np.log(2.0))

# Per-block jj-slots whose exp runs on DVE (Schraudolph) instead of ACT.
# Block 0's DVE budget is spent on V/QK chunk casts; later blocks give DVE
# more exp tiles. j=0 always stays on ACT (cross-block carry).
# Slots whose exp runs as ONE full-width ACT activation. All other slots
# SPLIT the tile across engines: ACT takes the h0 half, DVE-Schraudolph
# the h1 half, concurrently -- the scores PSUM tile then frees in ~0.75us
# instead of 1.15us, releasing the scores(j+2) WAR earlier (the st pool
# is 2-deep; this WAR cycle sets the slot cadence). Full-ACT slots exist
# where DVE is busy with casts (block 0-2) and to balance engine loads.
FULL_ACT_SETS = (
    frozenset(range(16)) - frozenset((6, 11)),
    frozenset(range(16)) - frozenset((2, 5, 8, 12)),
    frozenset(range(16)) - frozenset((2, 5, 8, 12)),
    frozenset(range(16)) - frozenset((2, 5, 8, 11, 14)),
    frozenset(range(16)) - frozenset((2, 5, 8, 11, 14)),
    frozenset(range(16)) - frozenset((1, 4, 7, 10, 13)),
    frozenset(range(16)) - frozenset((1, 4, 7, 10, 13)),
    frozenset(range(16)) - frozenset((1, 4, 7, 10, 13)),
)
# kept for block_plan availability logic: split tiles' DVE half lags ~1 slot
DVE_SETS = tuple(
    frozenset(jj for jj in range(16) if jj not in s) for s in FULL_ACT_SETS
)

def block_plan(bi):
    """Per-block exp/AV plan. ACT tiles pair into fp8 DoubleRow AV matmuls
    (adjacent jj, both non-DVE); leftovers are plain-bf16 ACT tiles. Returns
    (dve_set, fp8_pair_starts, act16_singles, av_ops) where av_ops is the AV
    consumption order [(avail_slot, kind, jj), ...]."""
    dset = DVE_SETS[bi]
    pairs, paired = [], set()
    jj = 0
    while jj < 15:
        if len(pairs) < MAX_FP8_PAIRS and jj not in dset and jj + 1 not in dset:
            pairs.append(jj)
            paired.update((jj, jj + 1))
            jj += 2
        else:
            jj += 1
    singles = frozenset(
        jj for jj in range(16) if jj not in dset and jj not in paired
    )
    ops = [(j0 + 1, "dr", j0) for j0 in pairs]
    ops += [(jj + 1, "bf", jj) for jj in sorted(dset)]
    ops += [(jj, "bf", jj) for jj in sorted(singles)]
    ops.sort(key=lambda t: t[0])
    return dset, frozenset(pairs), singles, ops

# exported for test harnesses
LAST_EXEC_TIME_NS = None
LAST_RESULT = None

_CACHED_NC = None


def _kernel_body(tc, xT_d, wq_d, wk_d, wv_d, wo_d, out_d):
    from contextlib import ExitStack

    nc = tc.nc
    with ExitStack() as ctx:
        consts = ctx.enter_context(tc.tile_pool(name="consts", bufs=1))
        # pt depth 16: exp(j) WAR-waits the AV readers of the slot it
        # recycles; at block boundaries the lagged AV stream runs ~10
        # iterations behind the exp stream, so 10 buffers head-of-line
        # blocked the Scalar queue for ~4us per boundary.
        ptp = ctx.enter_context(tc.tile_pool(name="pt", bufs=18))
        normp = ctx.enter_context(tc.tile_pool(name="norm", bufs=3))
        # fout depth 8: a tile is WAR-held until its out-DMA COMPLETES
        # (~2-3us incl completion latency); 4 bufs throttled the projection
        # drain to ~700ns/chunk (observed 784ns PE gap cascade in the tail)
        foutp = ctx.enter_context(tc.tile_pool(name="fout", bufs=8))
        # PSUM budget (8 banks): "s" 2x[128,1024]=4, "o" 3x[128,512]=3, "x" 1
        ps_s = ctx.enter_context(tc.tile_pool(name="ps_s", bufs=2, space="PSUM"))
        ps_o = ctx.enter_context(tc.tile_pool(name="ps_o", bufs=3, space="PSUM"))
        ps_x = ctx.enter_context(tc.tile_pool(name="ps_x", bufs=1, space="PSUM"))

        sb_xT = consts.tile([128, 4, S], BF16)  # X^T: k-chunk c -> [:, c, :]
        sb_wq = consts.tile([128, 4, DQ], BF16)
        sb_wk = consts.tile([128, 4, DQ], BF16)
        sb_wv = consts.tile([128, 4, DQ], BF16)
        sb_wo = consts.tile([128, 2, DM], BF16)  # d'-chunk p -> [:, p, :]
        sb_qT = consts.tile([128, 2, S], BF16)  # dq-chunk (head pair) p
        sb_kT = consts.tile([128, 2, S], BF16)
        sb_v = consts.tile([128, 16, HPC, 66], BF16)  # V_aug; col 64 = ones
        if USE_FP8:
            # fp8 copies for DoubleRow AV: per-jj [HPC, 68] (68 keeps the jj
            # stride 16B-aligned as DoubleRow's k-tile dim needs); col64=ones
            sb_v8 = consts.tile([128, 16, HPC, 68], FP8)
            # fp8 exp tiles by jj (ACT writes, DR-AV reads pairs)
            sb_pt8 = consts.tile([128, 16, 1024], FP8)
        else:
            sb_v8 = sb_pt8 = None
        # normalized O^T, one tile per head pair (separate tiles so the
        # dependency tracker never aliases pair-0 reads with pair-1 writes)
        sb_oT0 = consts.tile([128, S], BF16)
        sb_oT1 = consts.tile([128, S], BF16)
        sb_oT = (sb_oT0, sb_oT1)
        sb_warm = consts.tile([128, 512], BF16)  # PE warmup fodder
        sb_one = consts.tile([128, 64], F16)  # all-ones (bcast stationary)
        # Wo pair-1 h1 rows re-staged at partitions 0:64: lets the tail's
        # pair-1 projection read h1's normalized output straight from tmpb
        # (partitions 0:64) instead of waiting ~2.3us for the oT-upper
        # SBUF->SBUF DMA completion on the critical path
        sb_wo2 = consts.tile([64, 512], BF16)
        sb_shift = consts.tile([128, 1], F32)  # EXP_SHIFT bias for ACT exp

        nc.vector.memset(sb_shift[:], EXP_SHIFT)

        nc.vector.memset(sb_warm[:], 1.0)
        nc.vector.memset(sb_one[:], 1.0)
        nc.vector.memset(sb_v[:, :, :, 64:66], 1.0)
        if USE_FP8:
            nc.vector.memset(sb_v8[:, :, :, 64:65], 1.0)

        # DMA on two hardware queues so the transfers overlap: weights on
        # the Activation-queue DGE, X^T s-blocks on the Sync-queue DGE.
        # Ordered by first use; the lead is gated by wq/wk + s-block 0.
        # DMA trace 'durations' are descriptor-issue only -- the real data
        # movement runs long after (2MB of xT ~ 5.6us), and adjacent
        # dma_starts share one batched completion event. So: one dma per
        # tile, each emitted just before its FIRST consumer so the events
        # split per-transfer and early consumers don't wait on late DMAs.
        # DMA on two hardware queues so the transfers overlap: weights on
        # the Activation-queue DGE, X^T s-blocks on the Sync-queue DGE.
        # Ordered by first use; the lead is gated by wq/wk + s-block 0.
        xT_r = xT_d.rearrange("(c p) s -> p c s", p=128)
        nc.scalar.dma_start(sb_wq[:], wq_d.rearrange("(c p) d -> p c d", p=128))
        nc.scalar.dma_start(sb_wk[:], wk_d.rearrange("(c p) d -> p c d", p=128))
        nc.sync.dma_start(sb_xT[:, :, 0:256], xT_r[:, :, 0:256])
        nc.sync.dma_start(sb_xT[:, :, 256:512], xT_r[:, :, 256:512])
        for si in range(1, 4):
            isl = slice(si * 512, (si + 1) * 512)
            nc.sync.dma_start(sb_xT[:, :, isl], xT_r[:, :, isl])

        # Preload the exp table-set on ScalarE (the ~2.7us ACT_TABLE_LOAD
        # runs under the input DMA instead of gating exp0), then queue the
        # later-needed weights behind it on the same DGE.
        warm_act = normp.tile([1, 4], F32, tag="wact")
        nc.scalar.activation(warm_act[:], sb_warm[0:1, 0:4], AF.Exp, scale=-1.0)
        nc.scalar.dma_start(sb_wv[:], wv_d.rearrange("(c p) d -> p c d", p=128))
        nc.scalar.dma_start(sb_wo[:], wo_d.rearrange("(c p) d -> p c d", p=128))
        nc.scalar.dma_start(sb_wo2[:], wo_d[192:256, :])

        pw = ps_x.tile([128, 512], F32, tag="x")

        def warm_mm():
            nc.tensor.matmul(
                pw[:], lhsT=sb_warm[:, 0:128], rhs=sb_warm[:], start=True, stop=True
            )

        def emit_qk_chunk(w_sb, dst_sb, p, c, pool, tag):
            """One [128,512] chunk of Q^T or K^T for head-pair p."""
            isl = slice(c * 512, (c + 1) * 512)
            pq = pool.tile([128, 512], F32, tag=tag, name="pqk")
            for kc in range(4):
                nc.tensor.matmul(
                    pq[:],
                    lhsT=w_sb[:, kc, p * 128 : (p + 1) * 128],
                    rhs=sb_xT[:, kc, isl],
                    start=(kc == 0),
                    stop=(kc == 3),
                )
            nc.vector.tensor_copy(dst_sb[:, p, isl], pq[:])

        def emit_v_chunk(sc):
            """V natural [s,dv] for s-chunk sc (all 4 heads)."""
            pv = ps_x.tile([128, DQ], F32, tag="x", name="pv")
            for kc in range(4):
                nc.tensor.matmul(
                    pv[:],
                    lhsT=sb_xT[:, kc, sc * 128 : (sc + 1) * 128],
                    rhs=sb_wv[:, kc, :],
                    start=(kc == 0),
                    stop=(kc == 3),
                )
            nc.vector.tensor_copy(
                sb_v[:, sc, :, 0:64], pv.rearrange("p (h d) -> p h d", h=HPC)
            )
            if USE_FP8:  # fp8 copy for DoubleRow AV
                nc.vector.tensor_copy(sb_v8[:, sc, :, 0:64], sb_v[:, sc, :, 0:64])

        # ---- lead: Q^T chunk 0 and K^T chunk 0 for pair 0. A few warmup
        # matmuls run under the first DMAs; the lead chunk matmuls are
        # DMA-gated anyway, so more warmups would only push them out.
        # K chunk 0 is emitted in two column pieces: scores j=0 only read
        # kT[:, 0:128], so its exp fires before the rest of the chunk. ----
        warm_mm()
        warm_mm()
        warm_mm()

        def emit_q0_piece(c0, c1):
            pq = ps_o.tile([128, c1 - c0], F32, tag="o", name="pq0")
            for kc in range(4):
                nc.tensor.matmul(
                    pq[:],
                    lhsT=sb_wq[:, kc, 0:128],
                    rhs=sb_xT[:, kc, c0:c1],
                    start=(kc == 0),
                    stop=(kc == 3),
                )
            nc.vector.tensor_copy(sb_qT[:, 0, c0:c1], pq[:])

        def emit_k0_piece(c0, c1):
            pk = ps_s.tile([128, c1 - c0], F32, tag="s", name="pk0")
            for kc in range(4):
                nc.tensor.matmul(
                    pk[:],
                    lhsT=sb_wk[:, kc, 0:128],
                    rhs=sb_xT[:, kc, c0:c1],
                    start=(kc == 0),
                    stop=(kc == 3),
                )
            nc.vector.tensor_copy(sb_kT[:, 0, c0:c1], pk[:])

        # Q chunk 0 in two column halves (gated by the two s0 DMA halves)
        # with the first K piece in between; scores j=0 read kT[:, 0:128]
        # and all of qT chunk 0, so exp0 fires right after the Qb cast.
        # V chunks 0-1 follow: they fill the rest of the DMA-bound window
        # (the scheduler runs them around the stalled first scores).
        emit_q0_piece(0, 256)
        emit_k0_piece(0, 128)
        emit_k0_piece(128, 256)  # needs only s0's first half; casts pre-Qb
        emit_q0_piece(256, 512)
        emit_v_chunk(0)
        emit_v_chunk(1)

        # deferred Q/K chunks, one matmul per j-slot (deadlines: each chunk
        # must land before the carry scores / j-iteration that reads it)
        pending_qk = [(sb_wq, sb_qT, 0, 1)]
        for c in range(2, 4):
            pending_qk.append((sb_wq, sb_qT, 0, c))
        pending_qk.append((sb_wk, sb_kT, 1, 0))
        pending_qk.append((sb_wq, sb_qT, 1, 0))
        for c in range(1, 4):
            pending_qk.append((sb_wk, sb_kT, 1, c))
        for c in range(1, 4):
            pending_qk.append((sb_wq, sb_qT, 1, c))
        qk_state = {"chunk": None, "tile": None, "kc": 0}

        def step_pending_qk():
            stt = qk_state
            if stt["chunk"] is None:
                if not pending_qk:
                    return
                stt["chunk"] = pending_qk.pop(0)
                stt["tile"] = ps_x.tile([128, 512], F32, tag="x", name="pqk1")
                stt["kc"] = 0
            w_sb, dst_sb, p, c = stt["chunk"]
            nc.tensor.matmul(
                stt["tile"][:],
                lhsT=w_sb[:, stt["kc"], p * 128 : (p + 1) * 128],
                rhs=sb_xT[:, stt["kc"], c * 512 : (c + 1) * 512],
                start=(stt["kc"] == 0),
                stop=(stt["kc"] == 3),
            )
            stt["kc"] += 1
            if stt["kc"] == 4:
                nc.vector.tensor_copy(
                    dst_sb[:, p, c * 512 : (c + 1) * 512], stt["tile"][:]
                )
                stt["chunk"] = None

        # deferred output-projection chunks, one matmul per j-slot
        pending_proj = []
        proj_state = {"c2": None, "tile": None, "p": 0}

        def step_pending_proj(drain=False):
            stt = proj_state
            if stt["c2"] is None:
                if not pending_proj:
                    return
                stt["c2"] = pending_proj.pop(0)
                # mid-run: the single-bank "x" pool is fine (chunks are slots
                # apart). In the tail drain it WAR-serializes chunks ~800ns
                # apart -- use the freed 2-buffer "s" pool there instead.
                if drain:
                    # 3-deep rotation (s, s, o-free-slot) so chunk n+1's
                    # matmul never WAR-waits the previous chunk's copy
                    stt["dn"] = stt.get("dn", 0) + 1
                    if stt["dn"] % 3 == 0:
                        stt["tile"] = ps_o.tile(
                            [128, 512], F32, tag="o", name="pfd_o"
                        )
                    else:
                        stt["tile"] = ps_s.tile(
                            [128, 512], F32, tag="s", name="pfd"
                        )
                else:
                    stt["tile"] = ps_x.tile([128, 512], F32, tag="x", name="pf")
                stt["p"] = 0
            c2, pp = stt["c2"], stt["p"]
            if pp < 2:
                nc.tensor.matmul(
                    stt["tile"][:],
                    lhsT=sb_oT[pp][:, c2 * 128 : (c2 + 1) * 128],
                    rhs=sb_wo[:, pp, :],
                    start=(pp == 0),
                    stop=(pp == 1),
                )
                stt["p"] += 1
            else:
                fo = foutp.tile([128, 512], F16, tag="fo")
                # alternate copy engines so the tail drain's copies pipeline
                # in parallel instead of serializing on ACT
                stt["n"] = stt.get("n", 0) + 1
                if stt["n"] % 2:
                    nc.scalar.copy(fo[:], stt["tile"][:])
                else:
                    nc.vector.tensor_copy(fo[:], stt["tile"][:])
                nc.sync.dma_start(out_d[c2 * 128 : (c2 + 1) * 128, :], fo[:])
                stt["c2"] = None

        tail_tmpb = []

        # ---- normalization of a finished block (lazy, into next block) ----
        def make_norm_steps(p, ic, po, tail=False):
            isl = slice(ic * 512, (ic + 1) * 512)
            held = {}

            def step_sums(hi):
                s = normp.tile([65, 512], F16, tag="sums", name=f"sums{hi}")
                if tail and hi == 0:
                    # h0's sums on the now-idle ScalarE, h1's on DVE -- the
                    # two copies run in parallel so neither head's chain
                    # waits behind the other (Copy needs no table switch)
                    nc.scalar.copy(s[64:65, :], po[hi][64:65, :])
                else:
                    nc.vector.tensor_copy(s[64:65, :], po[hi][64:65, :])
                held[hi] = s

            def step_head(hi):
                pb = ps_x.tile([64, 512], F32, tag="x", name=f"pb{hi}")
                nc.tensor.matmul(
                    pb[:],
                    lhsT=sb_one[64:65, :],
                    rhs=held[hi][64:65, :],
                    start=True,
                    stop=True,
                )
                rec = normp.tile([64, 512], F32, tag="rec", name=f"rec{hi}")
                nc.vector.reciprocal_approx_fast(rec[:], pb[:])
                if hi == 0:
                    nc.vector.tensor_mul(sb_oT[p][0:64, isl], po[0][0:64, :], rec[:])
                else:
                    tmpb = normp.tile([64, 512], BF16, tag="tmpb")
                    nc.vector.tensor_mul(tmpb[:], po[1][0:64, :], rec[:])
                    if tail:
                        # no oT DMA: the tail projection reads tmpb directly
                        tail_tmpb.append(tmpb)
                    else:
                        nc.sync.dma_start(sb_oT[p][64:128, isl], tmpb[:])

            return step_sums, [lambda: step_head(0), lambda: step_head(1)]

        # ---- attention blocks ----
        blocks = [(p, ic) for p in range(2) for ic in range(4)]

        def emit_scores(p, ic, j):
            # high priority: the scheduler must never wedge deferred work
            # between the two concurrent row-tile matmuls or ahead of them
            # -- the exp stream (the critical engine) waits on both.
            isl = slice(ic * 512, (ic + 1) * 512)
            jsl = slice(j * 128, (j + 1) * 128)
            st = ps_s.tile([128, 1024], F32, tag="s")
            with tc.high_priority():
                nc.tensor.matmul(
                    st[:, 0:512],
                    lhsT=sb_kT[0:64, p, jsl],
                    rhs=sb_qT[0:64, p, isl],
                    start=True,
                    stop=True,
                )
                nc.tensor.matmul(
                    st[:, 512:1024],
                    lhsT=sb_kT[64:128, p, jsl],
                    rhs=sb_qT[64:128, p, isl],
                    start=True,
                    stop=True,
                )
            return st

        from concourse.alu_op_type import AluOpType

        def emit_exp(st, jj, mode):
            """exp of one scores tile. mode: 'act8' (ACT -> fp8 shared buf),
            'act16' (ACT -> bf16 pool tile), 'dve' (Schraudolph -> bf16)."""
            with tc.high_priority():
                if mode == "act8":
                    nc.scalar.activation(
                        sb_pt8[:, jj, :],
                        st[:],
                        AF.Exp,
                        scale=SCALE,
                        bias=sb_shift[:],
                    )
                    return None
                pt = ptp.tile([128, 1024], BF16, tag="pt")
                if mode == "dve":
                    # Schraudolph fast exp on DVE (whole tile)
                    nc.vector.tensor_scalar(
                        pt[:].bitcast(I16),
                        st[:],
                        A_EXP,
                        B_EXP,
                        AluOpType.mult,
                        AluOpType.add,
                    )
                elif USE_FP8:
                    nc.scalar.activation(
                        pt[:], st[:], AF.Exp, scale=SCALE, bias=sb_shift[:]
                    )
                else:
                    nc.scalar.activation(pt[:], st[:], AF.Exp, scale=SCALE)
                return pt

        pending_norm = []
        carried = False
        carry_tile = None
        for bi, (p, ic) in enumerate(blocks):
            po = [
                ps_o.tile([65, 512], F32, tag="o", name=f"po{hi}") for hi in range(2)
            ]
            pts = {}
            if carried:
                pts[0] = carry_tile  # previous block emitted our jj=0 exp
            dset, fp8_pairs, singles16, av_ops = block_plan(bi)

            def mode_of(jj, _s=singles16, _d=dset):
                return "dve" if jj in _d else ("act16" if jj in _s else "act8")

            def emit(jj, _p=p, _ic=ic, _pts=pts, _m=mode_of):
                _pts[jj] = emit_exp(emit_scores(_p, _ic, jj), jj, _m(jj))

            av_ptr = [0, 0]

            def emit_av(hi, _po=po, _ops=av_ops, _pts=pts, _p=p, _ptr=av_ptr):
                opi = _ptr[hi]
                _ptr[hi] += 1
                avail, kind, jj = _ops[opi]
                start, stop = opi == 0, opi == len(_ops) - 1
                if kind == "dr":
                    nc.tensor.matmul(
                        _po[hi][:],
                        lhsT=sb_v8[:, jj : jj + 2, 2 * _p + hi, 0:65],
                        rhs=sb_pt8[:, jj : jj + 2, hi * 512 : (hi + 1) * 512],
                        start=start,
                        stop=stop,
                        perf_mode=mybir.MatmulPerfMode.DoubleRow,
                        skip_group_check=True,
                    )
                else:
                    nc.tensor.matmul(
                        _po[hi][:],
                        lhsT=sb_v[:, jj, 2 * _p + hi, 0:65],
                        rhs=_pts[jj][:, hi * 512 : (hi + 1) * 512],
                        start=start,
                        stop=stop,
                        skip_group_check=True,
                    )

            def pump_av(hi, j, _ops=av_ops, _ptr=av_ptr):
                lag = 1 if hi == 0 else 2
                # catch up if fewer slots remain than ops outstanding
                n = max(1, (len(_ops) - _ptr[hi]) - (15 - j))
                while n and _ptr[hi] < len(_ops) and _ops[_ptr[hi]][0] <= j - lag:
                    emit_av(hi)
                    n -= 1

            def drain_av(hi, _ops=av_ops, _ptr=av_ptr):
                while _ptr[hi] < len(_ops):
                    emit_av(hi)

            if p == 1 and ic > 0:
                # previous ic's projection chunks; their oT inputs complete
                # during this block's first iterations (lazy norm)
                pending_proj.extend(range(4 * (ic - 1), 4 * ic))

            for j in range(16):
                # ready AV matmuls FIRST: they never stall, so the in-order
                # PE queue streams them while the upcoming scores matmul
                # WAR-waits on the st bank its exp-2-back is still reading.
                if j >= 2:
                    pump_av(0, j)
                    pump_av(1, j)
                if pending_norm and j == 1:
                    # both heads together: their bcast matmuls pipeline
                    # back-to-back instead of paying two isolated drains
                    pending_norm[0]()
                    pending_norm[1]()
                    pending_norm = []
                # deferred matmuls keep PE fed without starving ACT
                if bi == 0:
                    if j <= 1:
                        pass  # block-0 front work emitted with the scores
                    elif j < 4:
                        emit_qk_chunk(sb_wk, sb_kT, 0, j, ps_o, "o")
                        emit_v_chunk(j + 2)
                    else:
                        if j < 14:
                            emit_v_chunk(j + 2)
                        if j >= 5 and j % 2 == 1:
                            step_pending_qk()  # Q0 chunk 1 before the carry
                elif j >= 3:
                    # blocks 1-2 step at half rate so the deferred-chunk
                    # queue stretches through blocks 3-4 (keeps the PE
                    # activity window fed -- an idle PE re-throttles HAM)
                    if bi in (1, 2):
                        if j % 2 == 1:
                            step_pending_qk()
                    elif bi in (3, 4):
                        step_pending_qk()
                    else:
                        step_pending_proj()
                # scores + exp for this slot
                if j == 0 and carried:
                    carried = False  # exp emitted by the previous block
                elif bi == 0 and j == 0:
                    # pre-emit scores+exp for j=0..3 with the deferred work
                    # interleaved so each score lands in the in-order PE
                    # queue just before ACT needs it -- otherwise the K/V
                    # chunk matmuls bubble the exp stream at block-0 start
                    emit(0)
                    emit(1)
                    emit_k0_piece(256, 384)
                    emit_v_chunk(2)
                    emit(2)
                    emit_k0_piece(384, 512)
                    emit_qk_chunk(sb_wk, sb_kT, 0, 1, ps_o, "o")
                    emit_v_chunk(3)
                    emit(3)
                elif bi == 0 and j <= 3:
                    pass  # emitted at j=0
                else:
                    emit(j)
                if j == 15:
                    # cross-block pipeline: next block's first scores+exp
                    # go ahead of the AV tail so ACT never gaps.
                    last = bi + 1 == len(blocks)
                    if not last:
                        np_, nic = blocks[bi + 1]
                        nm = (
                            "dve"
                            if 0 in DVE_SETS[bi + 1]
                            else ("act8" if 0 in block_plan(bi + 1)[1] else "act16")
                        )
                        carry_tile = emit_exp(emit_scores(np_, nic, 0), 0, nm)
                        carried = True
                    step_sums, pending_norm = make_norm_steps(p, ic, po, tail=last)
                    drain_av(0)
                    step_sums(0)  # h0 sums right behind h0's last AV
                    drain_av(1)
                    step_sums(1)

        # ---- tail: the last 4 projection chunks' pair-0 matmuls first
        # (their oT inputs are long done), with a few warm matmuls to keep
        # the PE activity window fed through the norm chain; the last
        # block's norm (h1 first so its SBUF->SBUF DMA overlaps h0's DVE
        # chain); then the pair-1 matmuls, copies and output DMAs. ----
        while pending_proj or proj_state["c2"] is not None:
            step_pending_proj(drain=True)
        pfs = []
        # pf tiles mostly on the "s" pool: its slots freed when the last
        # exp drained; "o" slots are still WAR-held by the last block's po
        # norm reads, which serialized the pair-1 matmuls ~800ns apart.
        for n, c2 in enumerate(range(12, 16)):
            pool, tag = (ps_o, "o") if n == 1 else (ps_s, "s")
            pf = pool.tile([128, 512], F32, tag=tag, name=f"pfz{n}")
            nc.tensor.matmul(
                pf[:],
                lhsT=sb_oT[0][:, c2 * 128 : (c2 + 1) * 128],
                rhs=sb_wo[:, 0, :],
                start=True,
                stop=False,
                skip_group_check=True,
            )
            pfs.append((c2, pf))
        pending_norm[1]()  # head 1 (mul -> tmpb, read directly below)
        pending_norm[0]()  # head 0 (direct mul)
        # keep the PE activity window fed through the norm chain (else HAM
        # re-throttles and the pair-1 matmuls run at half clock); one warm
        # on the free "s" slot, one on "x" (serialized after the norm
        # broadcast's reader, so it fills the pre-projection idle window)
        ws = ps_s.tile([128, 512], F32, tag="s", name="warm_s")
        nc.tensor.matmul(
            ws[:], lhsT=sb_warm[:, 0:128], rhs=sb_warm[:], start=True, stop=True
        )
        warm_mm()
        # pair-1 contraction as two sequential K=64 matmuls (same base
        # partition -- no row-tile concurrency, so no PSUM race): the h0
        # rows come from sb_oT[1] (direct DVE mul), the h1 rows straight
        # from tmpb at partitions 0:64 against the re-staged sb_wo2.
        # Copy + DMA chase each chunk.
        tb = tail_tmpb[0]
        for n, (c2, pf) in enumerate(pfs):
            lc = (c2 - 12) * 128
            nc.tensor.matmul(
                pf[:],
                lhsT=sb_oT[1][0:64, c2 * 128 : (c2 + 1) * 128],
                rhs=sb_wo[0:64, 1, :],
                start=False,
                stop=False,
                skip_group_check=True,
            )
            nc.tensor.matmul(
                pf[:],
                lhsT=tb[0:64, lc : lc + 128],
                rhs=sb_wo2[:],
                start=False,
                stop=True,
                skip_group_check=True,
            )
            fo = foutp.tile([128, 512], F16, tag="fo")
            # split the copies across ACT and DVE so the tail drains fast
            if n % 2 == 0:
                nc.scalar.copy(fo[:], pf[:])
            else:
                nc.vector.tensor_copy(fo[:], pf[:])
            eng = nc.sync if n % 2 == 0 else nc.scalar
            eng.dma_start(out_d[c2 * 128 : (c2 + 1) * 128, :], fo[:])


def _build():
    nc = bacc.Bacc("TRN2", target_bir_lowering=False, debug=False, num_devices=N_CORES)
    xT = nc.dram_tensor("xT", [DM, S], BF16, kind="ExternalInput")
    wq = nc.dram_tensor("wq", [DM, DQ], BF16, kind="ExternalInput")
    wk = nc.dram_tensor("wk", [DM, DQ], BF16, kind="ExternalInput")
    wv = nc.dram_tensor("wv", [DM, DQ], BF16, kind="ExternalInput")
    wo = nc.dram_tensor("wo", [DQ, DM], BF16, kind="ExternalInput")
    out = nc.dram_tensor("out", [S, DM], F16, kind="ExternalOutput")
    with tile.TileContext(nc) as tc:
        _kernel_body(tc, xT.ap(), wq.ap(), wk.ap(), wv.ap(), wo.ap(), out.ap())
    nc.compile()
    return nc


def get_nc():
    global _CACHED_NC
    if _CACHED_NC is None:
        _CACHED_NC = _build()
    return _CACHED_NC


def _in_maps(hidden_states, Wq, Wk, Wv, Wo):
    bf = ml_dtypes.bfloat16
    maps = []
    for c in range(N_CORES):
        b, g = c // 2, c % 2
        cols = slice(g * DQ, (g + 1) * DQ)
        maps.append(
            {
                "xT": np.ascontiguousarray(hidden_states[b].T).astype(bf),
                "wq": np.ascontiguousarray(Wq[:, cols]).astype(bf),
                "wk": np.ascontiguousarray(Wk[:, cols]).astype(bf),
                "wv": np.ascontiguousarray(Wv[:, cols]).astype(bf),
                "wo": np.ascontiguousarray(Wo[cols, :]).astype(bf),
            }
        )
    return maps


def _ensure_profile_support():
    """Best-effort: register the axon NTFF profiling hook + defang the
    bucket upload (zero-egress container). Without this, trace=True dies
    on a missing ``antenv.axon_hooks`` module in this image."""
    import types

    try:
        import antenv

        if "antenv.axon_hooks" not in sys.modules:
            mod = types.ModuleType("antenv.axon_hooks")
            _h = {"hook": None}
            mod.set_axon_ntff_profile_hook = lambda h: _h.__setitem__("hook", h)
            mod.get_axon_ntff_profile_hook = lambda: _h["hook"]
            sys.modules["antenv.axon_hooks"] = mod
            antenv.axon_hooks = mod
        import antenv.axon_hooks as ah

        if ah.get_axon_ntff_profile_hook() is None:
            if "/root/.axon_site" not in sys.path:
                sys.path.append("/root/.axon_site")
            from trn_agent_boot.trn_boot import _ntff_profile_via_ctypes

            hook = _ntff_profile_via_ctypes("/opt/axon/libaxon_pjrt.so")
            if hook is not None:
                ah.set_axon_ntff_profile_hook(hook)
    except Exception:
        pass
    try:
        import concourse.bass_utils as bu

        bu.upload_artifacts = lambda tmpdir: tmpdir
    except Exception:
        pass


def kernel(hidden_states, Wq, Wk, Wv, Wo):
    global LAST_EXEC_TIME_NS, LAST_RESULT
    hidden_states = np.asarray(hidden_states, dtype=np.float32)
    Wq, Wk, Wv, Wo = (np.asarray(w, dtype=np.float32) for w in (Wq, Wk, Wv, Wo))

    trace = bool(os.environ.get("BASS_TRACE"))
    if trace:
        _ensure_profile_support()
    nc = get_nc()
    maps = _in_maps(hidden_states, Wq, Wk, Wv, Wo)
    res = run_bass_kernel_spmd(
        nc,
        maps,
        core_ids=list(range(N_CORES)),
        trace=trace,
        tmpdir=os.environ.get("BASS_TRACE_DIR") or None,
    )
    LAST_RESULT = res
    LAST_EXEC_TIME_NS = res.exec_time_ns

    out = np.empty((B, S, DM), dtype=np.float32)
    for b in range(B):
        out[b] = res.results[2 * b]["out"].astype(np.float32) + res.results[
            2 * b + 1
        ]["out"].astype(np.float32)
    return out


if __name__ == "__main__":
    rng = np.random.default_rng(0)
    hs = rng.standard_normal((B, S, DM), dtype=np.float32)
    ws = [
        (rng.standard_normal((DM, DM), dtype=np.float32) / np.sqrt(DM))
        for _ in range(4)
    ]
    o = kernel(hs, *ws)
    print("out", o.shape, o.dtype, float(np.abs(o).mean()))
    print("exec_time_ns", LAST_EXEC_TIME_NS)

